# revision 1
# baseline (speedup 1.0000x reference)
"""Submanifold 3x3x3 sparse conv (gnn_message_passing) + BatchNorm + LeakyReLU
on 8 Trainium2 NeuronCores.

Strategy (hardcoded for N=200000, C=128, K=27, GRID=128^3 @ ~9.5% occupancy):
  * The active-voxel neighbor graph at this occupancy is far below the cubic
    site-percolation threshold, so it splits into ~31k tiny connected
    components (max ~2.4k voxels). We partition whole components across the
    8 cores (LPT bin packing) -> every neighbor reference stays inside its
    core's shard. No halo exchange, and shard-local indices fit in int16,
    which is what the SWDGE dma_gather ucode requires.
  * Per core: the shard's features live as a bf16 token table in SBUF.
    For each 512-row supertile, ONE merged dma_gather (transpose=True)
    gathers all 26 non-self neighbor rows k-major -> a [128, 26*512] bf16
    tile that is directly the transposed matmul rhs. The self offset (k=13)
    is a contiguous slice streamed from a host-pretransposed table.
  * 27 accumulating bf16 matmuls (lhsT = W[k], C_in on partitions) produce
    the conv output transposed [C_out, 512] in fp32 PSUM.
  * BN statistics: per-supertile DVE reduce (sum) + ACT Square with
    accum_out (sum of squares), finalized and all-reduced across the 8
    cores with one tiny AllReduce collective. b is ignored: BatchNorm is
    shift-invariant so the conv bias cancels exactly.
  * BN apply + LeakyReLU is a single ACT Lrelu instruction per tile
    (out = lrelu(x*scale + shift), per-partition scale/shift), then PE
    transposes back to row-major and contiguous DMA writeback.
  * Host reassembles shards and inverts the component permutation.

Falls back to a pure-numpy reference computation if the input graph is not
separable into <=25088-row shards (never the case for the intended input
distribution).
"""

import numpy as np
import ml_dtypes

C = 128
K = 27
EPS = 1e-4
LEAK = 0.333
N_CORES = 8
SELF_K = 13

F32 = None  # set lazily after concourse import
BF16 = None
I16 = None


class Cfg:
    def __init__(self, n_total, st, n_st, table_rows, n_cores):
        assert st % 128 == 0 and table_rows % 128 == 0
        self.n_total = n_total          # global number of real rows (stats divisor)
        self.st = st                    # supertile rows
        self.n_st = n_st                # supertiles per core
        self.shard = st * n_st          # padded rows per core
        self.table_rows = table_rows    # shard table rows incl. zero pad
        self.ranks = table_rows // 128
        self.zero_row = table_rows - 1
        self.n_cores = n_cores
        self.kg = K - 1                 # gathered (non-self) offsets
        self.merged = self.kg * st      # idxs per merged gather
        self.idx_cols = self.merged // 16
        assert self.merged % 128 == 0
        # Chunked single-packet gathers: 512 idxs = 32 descs/engine (8KB
        # packet) verified working; 1024 idxs (16KB packet) crashes the
        # exec unit; single_packet=False (per-desc packets) is ~2x slower
        # end-to-end. HW-measured 2026-08-04.
        self.gather_chunk = 512 if self.merged % 512 == 0 else (
            256 if self.merged % 256 == 0 else 0)


FULL_CFG = Cfg(n_total=200_000, st=512, n_st=49, table_rows=25_216, n_cores=N_CORES)


def emit_kernel(tc, out_ap, ins, cfg):
    """Emit the per-core program. ins: dict with APs for
    table [table_rows, C] bf16, table_t [C, shard] bf16,
    idx [128, n_st*idx_cols] int16, w [K, C, C] bf16,
    gamma [C] f32, beta [C] f32. out_ap: [shard, C] f32."""
    import concourse.mybir as mybir
    from concourse.bass import ts
    from concourse.masks import make_identity

    nc = tc.nc
    F32 = mybir.dt.float32
    BF16 = mybir.dt.bfloat16
    I16 = mybir.dt.int16
    ST, N_ST = cfg.st, cfg.n_st
    NB = ST // 128  # row blocks per supertile

    table, table_t, idx, w = ins["table"], ins["table_t"], ins["idx"], ins["w"]
    gamma, beta = ins["gamma"], ins["beta"]

    # order of gathered offsets in the merged index list
    kg_list = [k for k in range(K) if k != SELF_K]

    with (
        tc.tile_pool(name="const", bufs=1) as constp,
        tc.tile_pool(name="gath", bufs=2) as gathp,
        tc.tile_pool(name="selfp", bufs=2) as selfp,
        tc.tile_pool(name="idxp", bufs=2) as idxp,
        tc.tile_pool(name="work", bufs=2) as workp,
        tc.tile_pool(name="psum", bufs=2, space="PSUM") as psump,
        tc.tile_pool(name="psumt", bufs=2, space="PSUM") as psumtp,
        tc.tile_pool(name="dram", bufs=1, space="DRAM") as dramp,
    ):
        table_sb = constp.tile([128, cfg.ranks * C], BF16)
        nc.sync.dma_start(table_sb[:].rearrange("p (r c) -> p r c", r=cfg.ranks),
                          table.rearrange("(r p) c -> p r c", p=128))
        w_sb = constp.tile([128, K * C], BF16)
        nc.sync.dma_start(w_sb[:].rearrange("ci (k co) -> ci k co", k=K),
                          w.rearrange("k ci co -> ci k co"))
        gamma_sb = constp.tile([128, 1], F32)
        nc.sync.dma_start(gamma_sb[:], gamma[:, None])
        beta_sb = constp.tile([128, 1], F32)
        nc.sync.dma_start(beta_sb[:], beta[:, None])
        identity = constp.tile([128, 128], F32)
        make_identity(nc, identity[:])

        out_t = constp.tile([128, cfg.shard], BF16)   # staged pre-BN, transposed
        sum_part = constp.tile([128, N_ST], F32)
        sq_part = constp.tile([128, N_ST], F32)
        gt_probe = None
        if getattr(cfg, "skip_gather", False):  # perf probe only
            gt_probe = constp.tile([128, 1, cfg.merged], BF16)
            nc.vector.memset(gt_probe[:, :, :], 0)

        # ---- phase 1: conv + stats ----
        for s in range(N_ST):
            it = idxp.tile([128, cfg.idx_cols], I16)
            nc.sync.dma_start(it[:], idx[:, s * cfg.idx_cols:(s + 1) * cfg.idx_cols])
            chunk = getattr(cfg, "gather_chunk", 0)
            if gt_probe is not None:  # perf probe only
                gt = gt_probe
            else:
                gt = gathp.tile([128, 1, cfg.merged], BF16)
            if gt_probe is not None:
                pass
            elif chunk:
                # single_packet=True needs <=1024 idxs (64 descs/engine =
                # one packet); chunked gathers keep packets maximal.
                assert cfg.merged % chunk == 0 and chunk <= 1024
                cw = chunk // 16
                for gc in range(cfg.merged // chunk):
                    nc.gpsimd.dma_gather(
                        gt[:, :, gc * chunk:(gc + 1) * chunk], table_sb[:],
                        it[:, gc * cw:(gc + 1) * cw], chunk, chunk, C,
                        transpose=True,
                        single_packet=True,
                        sbuf_tokens_per_rank=128,
                        sbuf_free_dim_per_rank=C * 2,
                    )
            else:
                nc.gpsimd.dma_gather(
                    gt[:, :, :], table_sb[:], it[:], cfg.merged, cfg.merged, C,
                    transpose=True,
                    single_packet=False,
                    sbuf_tokens_per_rank=128,
                    sbuf_free_dim_per_rank=C * 2,
                )
            st_self = selfp.tile([128, ST], BF16)
            nc.sync.dma_start(st_self[:], table_t[:, s * ST:(s + 1) * ST])

            ps = psump.tile([128, ST], F32)
            for kk in range(K):
                if kk == SELF_K:
                    rhs = st_self[:]
                else:
                    kidx = kg_list.index(kk)
                    rhs = gt[:, 0, ts(kidx, ST)]
                nc.tensor.matmul(ps[:], w_sb[:, ts(kk, C)], rhs,
                                 start=(kk == 0), stop=(kk == K - 1))

            nc.vector.reduce_sum(out=sum_part[:, s:s + 1], in_=ps[:],
                                 axis=mybir.AxisListType.X)
            trash = workp.tile([128, ST], F32)
            nc.scalar.activation(trash[:], ps[:],
                                 mybir.ActivationFunctionType.Square,
                                 accum_out=sq_part[:, s:s + 1])
            nc.vector.tensor_copy(out_t[:, s * ST:(s + 1) * ST], ps[:])

        # ---- stats finalize + all-reduce ----
        stats_sb = constp.tile([128, 2], F32)
        nc.vector.reduce_sum(out=stats_sb[:, 0:1], in_=sum_part[:],
                             axis=mybir.AxisListType.X)
        nc.vector.reduce_sum(out=stats_sb[:, 1:2], in_=sq_part[:],
                             axis=mybir.AxisListType.X)

        if cfg.n_cores > 1 and not getattr(cfg, "skip_collective", False):
            stats_in = dramp.tile([128, 2], F32)
            stats_out = dramp.tile([128, 2], F32)
            nc.sync.dma_start(stats_in[:], stats_sb[:])
            nc.gpsimd.collective_compute(
                "AllReduce", mybir.AluOpType.add,
                replica_groups=[list(range(cfg.n_cores))],
                ins=[stats_in.opt()], outs=[stats_out.opt()],
            )
            stats2_sb = constp.tile([128, 2], F32)
            nc.sync.dma_start(stats2_sb[:], stats_out[:])
        else:
            stats2_sb = stats_sb

        mean_t = constp.tile([128, 1], F32)
        ex2_t = constp.tile([128, 1], F32)
        var_t = constp.tile([128, 1], F32)
        std_t = constp.tile([128, 1], F32)
        rstd_t = constp.tile([128, 1], F32)
        s_vec = constp.tile([128, 1], F32)
        t_vec = constp.tile([128, 1], F32)
        tmp = constp.tile([128, 1], F32)
        inv_n = 1.0 / cfg.n_total
        nc.vector.tensor_scalar_mul(mean_t[:], stats2_sb[:, 0:1], inv_n)
        nc.vector.tensor_scalar_mul(ex2_t[:], stats2_sb[:, 1:2], inv_n)
        nc.vector.tensor_tensor(out=tmp[:], in0=mean_t[:], in1=mean_t[:],
                                op=mybir.AluOpType.mult)
        nc.vector.tensor_tensor(out=var_t[:], in0=ex2_t[:], in1=tmp[:],
                                op=mybir.AluOpType.subtract)
        nc.vector.tensor_scalar_add(var_t[:], var_t[:], EPS)
        nc.scalar.activation(std_t[:], var_t[:],
                             mybir.ActivationFunctionType.Sqrt)
        nc.vector.reciprocal(rstd_t[:], std_t[:])
        nc.vector.tensor_tensor(out=s_vec[:], in0=rstd_t[:], in1=gamma_sb[:],
                                op=mybir.AluOpType.mult)
        nc.vector.tensor_tensor(out=tmp[:], in0=mean_t[:], in1=s_vec[:],
                                op=mybir.AluOpType.mult)
        nc.vector.tensor_tensor(out=t_vec[:], in0=beta_sb[:], in1=tmp[:],
                                op=mybir.AluOpType.subtract)

        # ---- phase 2: BN + LeakyReLU + transpose back + writeback ----
        for s in range(N_ST):
            bn = workp.tile([128, ST], F32)
            nc.scalar.activation(bn[:], out_t[:, s * ST:(s + 1) * ST],
                                 mybir.ActivationFunctionType.Identity,
                                 bias=t_vec[:, 0:1], scale=s_vec[:, 0:1])
            bn2 = workp.tile([128, ST], F32)
            nc.vector.tensor_scalar_mul(bn2[:], bn[:], LEAK)
            nc.vector.tensor_tensor(out=bn[:], in0=bn[:], in1=bn2[:],
                                    op=mybir.AluOpType.max)
            pt = psumtp.tile([128, ST], F32)
            for b in range(NB):
                nc.tensor.transpose(pt[:, ts(b, 128)], bn[:, ts(b, 128)],
                                    identity[:])
            stage = workp.tile([128, ST], F32)
            nc.vector.tensor_copy(stage[:], pt[:])
            nc.sync.dma_start(
                out_ap[s * ST:(s + 1) * ST, :].rearrange("(b p) c -> p b c", p=128),
                stage[:].rearrange("p (b c) -> p b c", b=NB),
            )


# ----------------------------------------------------------------------------
# host-side preparation
# ----------------------------------------------------------------------------

def _partition_components(nb, n, n_cores, shard_cap):
    """Whole-component LPT partition. Returns (members_per_core, ok)."""
    import scipy.sparse as sp
    import scipy.sparse.csgraph as csg
    import heapq

    valid = nb >= 0
    ii, kk = np.nonzero(valid)
    jj = nb[ii, kk]
    m = kk != SELF_K
    g = sp.coo_matrix((np.ones(m.sum(), np.int8), (ii[m], jj[m])), shape=(n, n))
    _, labels = csg.connected_components(g, directed=False)
    sizes = np.bincount(labels)
    if sizes.max() > shard_cap:
        return None, False
    order = np.argsort(sizes)[::-1]
    heap = [(0, c) for c in range(n_cores)]
    heapq.heapify(heap)
    assign = np.empty(len(sizes), np.int32)
    for comp in order:
        load, c = heapq.heappop(heap)
        assign[comp] = c
        heapq.heappush(heap, (load + int(sizes[comp]), c))
    if max(l for l, _ in heap) > shard_cap:
        return None, False
    shard_of = assign[labels]
    members = [np.nonzero(shard_of == c)[0] for c in range(n_cores)]
    return members, True


def _prepare_core_inputs(features, nb, members, cfg):
    n = features.shape[0]
    loc = np.full(n, cfg.zero_row, np.int32)
    for mem in members:
        loc[mem] = np.arange(len(mem), dtype=np.int32)

    kg_list = [k for k in range(K) if k != SELF_K]
    in_maps = []
    for mem in members:
        real = len(mem)
        assert real <= cfg.shard
        table = np.zeros((cfg.table_rows, C), ml_dtypes.bfloat16)
        table[:real] = features[mem].astype(ml_dtypes.bfloat16)
        table_t = np.ascontiguousarray(
            table[:cfg.shard].T)  # [C, shard] bf16

        idx16 = np.full((cfg.shard, cfg.kg), cfg.zero_row, np.int32)
        nb_c = nb[mem][:, kg_list]                   # [real, kg]
        v = nb_c >= 0
        li = loc[np.where(v, nb_c, 0)]
        assert (li[v] < real).all(), "neighbor escaped shard"
        idx16[:real] = np.where(v, li, cfg.zero_row)
        idx16 = idx16.astype(np.int16)

        idx_dram = np.empty((128, cfg.n_st * cfg.idx_cols), np.int16)
        for s in range(cfg.n_st):
            m = idx16[s * cfg.st:(s + 1) * cfg.st, :].T.reshape(-1)  # k-major
            wrapped = m.reshape(cfg.idx_cols, 16).T                  # [16, cols]
            idx_dram[:, s * cfg.idx_cols:(s + 1) * cfg.idx_cols] = \
                np.tile(wrapped, (8, 1))
        in_maps.append({"table": table, "table_t": table_t, "idx": idx_dram})
    return in_maps


def _reference_fallback(features, w, b, gamma, beta, nb):
    feats = np.asarray(features, np.float32)
    wf = np.asarray(w, np.float32)
    out = np.broadcast_to(np.asarray(b, np.float32), feats.shape).copy()
    valid = nb >= 0
    idx = np.where(valid, nb, 0)
    for k in range(K):
        xk = feats[idx[:, k]] * valid[:, k:k + 1]
        out += xk @ wf[k]
    mean = out.mean(0)
    var = out.var(0)
    out = (out - mean) / np.sqrt(var + EPS) * np.asarray(gamma, np.float32) \
        + np.asarray(beta, np.float32)
    return np.where(out > 0, out, LEAK * out).astype(np.float32)


def _build_bass(cfg, reps=1):
    import concourse.bacc as bacc
    import concourse.mybir as mybir
    import concourse.tile as tile

    nc = bacc.Bacc("TRN2", target_bir_lowering=False, debug=False,
                   num_devices=cfg.n_cores)
    F32 = mybir.dt.float32
    BF16 = mybir.dt.bfloat16
    I16 = mybir.dt.int16
    ins = {
        "table": nc.dram_tensor("table", [cfg.table_rows, C], BF16,
                                kind="ExternalInput")[:, :],
        "table_t": nc.dram_tensor("table_t", [C, cfg.shard], BF16,
                                  kind="ExternalInput")[:, :],
        "idx": nc.dram_tensor("idx", [128, cfg.n_st * cfg.idx_cols], I16,
                              kind="ExternalInput")[:, :],
        "w": nc.dram_tensor("w", [K, C, C], BF16, kind="ExternalInput")[:, :, :],
        "gamma": nc.dram_tensor("gamma", [C], F32, kind="ExternalInput")[:],
        "beta": nc.dram_tensor("beta", [C], F32, kind="ExternalInput")[:],
    }
    out = nc.dram_tensor("out", [cfg.shard, C], F32, kind="ExternalOutput")
    with tile.TileContext(nc) as tc:
        for _ in range(reps):
            emit_kernel(tc, out[:, :], ins, cfg)
    nc.compile()
    return nc


def kernel(features, W, b, gamma, beta, neighbor_idx):
    from concourse.bass_utils import run_bass_kernel_spmd

    features = np.asarray(features, np.float32)
    Wf = np.asarray(W, np.float32)
    gamma_f = np.asarray(gamma, np.float32)
    beta_f = np.asarray(beta, np.float32)
    nb = np.asarray(neighbor_idx, np.int32)
    cfg = FULL_CFG
    assert features.shape == (cfg.n_total, C)

    members, ok = _partition_components(nb, cfg.n_total, cfg.n_cores, cfg.shard)
    if not ok:
        return _reference_fallback(features, Wf, b, gamma_f, beta_f, nb)

    core_maps = _prepare_core_inputs(features, nb, members, cfg)
    w_bf = Wf.astype(ml_dtypes.bfloat16)
    for m in core_maps:
        m["w"] = w_bf
        m["gamma"] = gamma_f
        m["beta"] = beta_f

    nc = _build_bass(cfg)
    res = run_bass_kernel_spmd(nc, core_maps, core_ids=list(range(cfg.n_cores)))

    out_full = np.empty((cfg.n_total, C), np.float32)
    for c, mem in enumerate(members):
        out_full[mem] = res.results[c]["out"][:len(mem)]
    return out_full



# revision 10
# speedup vs baseline: 5.8342x; 5.8342x over previous
"""Submanifold 3x3x3 sparse conv (gnn_message_passing) + BatchNorm + LeakyReLU
on 8 Trainium2 NeuronCores.

Strategy (hardcoded for N=200000, C=128, K=27, GRID=128^3 @ ~9.5% occupancy):
  * The active-voxel neighbor graph at this occupancy splits into ~31k tiny
    connected components (max ~2.4k voxels). Whole components are partitioned
    across the 8 cores (LPT bin packing) -> every neighbor reference stays
    inside its core's shard; shard-local indices fit in int16, required by
    the SWDGE dma_gather ucode.
  * At 9.5% occupancy only ~2.44 of the 26 non-self neighbor slots are
    active per voxel.  Instead of gathering all 26 rows per voxel (the
    SWDGE-bound dense scheme), each 512-row supertile gathers ONLY the
    valid (slot, k) pairs, k-major, into a compact [128, P] bf16 tile
    (~1.5k tokens instead of 13.3k).  Per-(tile,k) group widths are maxed
    across the 8 cores so one SPMD program serves all shards; shards pad
    their groups with zero-row tokens.
  * Step 1: per-k-range matmuls (lhsT = gathered columns, rhs = W[k])
    produce Zt [pairs, C_out] in PSUM, copied to SBUF as bf16 by ACT.
  * Step 2: the scatter-accumulate out[:, slot] += Zt[p, :] for
    slot = slot_of[p] is a matmul against a 0/1 indicator matrix
    Ind[p, slot] = (slot_of[p] == slot), generated on DVE per 128-pair
    chunk via tensor_scalar is_equal against a host-uploaded iota row.
    The self offset (k=13) is one dense matmul from a host-pretransposed
    table slice.  All accumulate into fp32 PSUM [C_out, 512].
  * Gathers are issued in <=256-idx single-packet chunks: SWDGE cost is
    ~0.7us per instruction nearly independent of idx count up to 256
    (512-idx chunks hit a ~6us/instruction pathology; HW-measured
    2026-08-08).
  * BN statistics: per-supertile DVE reduce (sum) + ACT Square with
    accum_out (sum of squares), finalized and all-reduced across the 8
    cores with one tiny AllReduce collective. b is ignored: BatchNorm is
    shift-invariant so the conv bias cancels exactly.
  * BN apply + LeakyReLU per tile, then PE transposes back to row-major
    and contiguous DMA writeback. Host reassembles shards and inverts the
    component permutation.

Falls back to the dense-gather kernel if the valid-pair structure exceeds
the sparse path's PSUM budget, and to pure numpy if the graph is not
separable into <=25088-row shards (never the case for the intended input
distribution).
"""

import numpy as np
import ml_dtypes

C = 128
K = 27
EPS = 1e-4
LEAK = 0.333
N_CORES = 8
SELF_K = 13
KG_LIST = [k for k in range(K) if k != SELF_K]


class Cfg:
    def __init__(self, n_total, st, n_st, table_rows, n_cores):
        assert st % 128 == 0 and table_rows % 128 == 0
        self.n_total = n_total          # global number of real rows (stats divisor)
        self.st = st                    # supertile rows
        self.n_st = n_st                # supertiles per core
        self.shard = st * n_st          # padded rows per core
        self.table_rows = table_rows    # shard table rows incl. zero pad
        self.ranks = table_rows // 128
        self.zero_row = table_rows - 1
        self.n_cores = n_cores
        self.kg = K - 1                 # gathered (non-self) offsets
        self.merged = self.kg * st      # idxs per merged gather (dense path)
        self.idx_cols = self.merged // 16
        assert self.merged % 128 == 0
        # SWDGE gathers: ~0.7us fixed cost per instruction for <=256 idxs;
        # 512-idx single-packet chunks are ~9x slower per idx (HW-measured
        # 2026-08-08). Keep chunks at 256.
        self.gather_chunk = 256 if self.merged % 256 == 0 else (
            128 if self.merged % 128 == 0 else 0)


FULL_CFG = Cfg(n_total=200_000, st=512, n_st=49, table_rows=25_216, n_cores=N_CORES)

MAX_PAIR_CHUNKS = 16   # Zt PSUM budget: 16 chunks * 512B = 4 banks


class TilePlan:
    """Per-supertile static structure, common to all cores (SPMD)."""
    __slots__ = ("phat", "n_chunks", "gchunks", "pieces")

    def __init__(self, phat, gchunks, pieces):
        self.phat = phat                  # padded pair count (x128)
        self.n_chunks = phat // 128       # 128-pair Ind/Zt chunks
        self.gchunks = gchunks            # gather chunk sizes (<=256, x128)
        self.pieces = pieces              # [(k, p0, p1)] matmul pieces


def build_plan(nb, members, cfg):
    """Common (max-over-cores) per-tile pair structure. Returns
    (plans, widths) where widths[s] = per-k group widths."""
    plans, widths = [], []
    valid = [nb[mem][:, KG_LIST] >= 0 for mem in members]  # [real, 26] each
    for s in range(cfg.n_st):
        r0, r1 = s * cfg.st, (s + 1) * cfg.st
        w = np.zeros(cfg.kg, np.int64)
        for v in valid:
            tile = v[r0:r1]
            if tile.shape[0]:
                np.maximum(w, tile.sum(0), out=w)
        total = int(w.sum())
        phat = max(128, -(-total // 128) * 128)
        if phat // 128 > MAX_PAIR_CHUNKS:
            return None, None
        w[-1] += phat - total  # pad tail into last group
        n128 = phat // 128
        gchunks = [256] * (n128 // 2) + [128] * (n128 % 2)
        # matmul pieces: ZtT free-axis ranges, split at 512-col PSUM banks
        pieces = []
        p = 0
        for ki, wk in enumerate(w):
            k = KG_LIST[ki]
            left = int(wk)
            while left:
                take = min(left, 512 - (p % 512))
                pieces.append((k, p, p + take))
                p += take
                left -= take
        assert p == phat
        plans.append(TilePlan(phat, gchunks, pieces))
        widths.append(w)
    return plans, widths


def _emit_bn_finalize(nc, tc, constp, dramp, cfg, stats_sb, gamma_sb, beta_sb):
    """All-reduce raw sums, produce per-channel scale/shift vectors."""
    import concourse.mybir as mybir
    F32 = mybir.dt.float32

    if cfg.n_cores > 1:
        stats_in = dramp.tile([128, 2], F32)
        stats_out = dramp.tile([128, 2], F32)
        nc.sync.dma_start(stats_in[:], stats_sb[:])
        nc.gpsimd.collective_compute(
            "AllReduce", mybir.AluOpType.add,
            replica_groups=[list(range(cfg.n_cores))],
            ins=[stats_in.opt()], outs=[stats_out.opt()],
        )
        stats2_sb = constp.tile([128, 2], F32)
        nc.sync.dma_start(stats2_sb[:], stats_out[:])
    else:
        stats2_sb = stats_sb

    mean_t = constp.tile([128, 1], F32)
    ex2_t = constp.tile([128, 1], F32)
    var_t = constp.tile([128, 1], F32)
    std_t = constp.tile([128, 1], F32)
    rstd_t = constp.tile([128, 1], F32)
    s_vec = constp.tile([128, 1], F32)
    t_vec = constp.tile([128, 1], F32)
    tmp = constp.tile([128, 1], F32)
    inv_n = 1.0 / cfg.n_total
    nc.vector.tensor_scalar_mul(mean_t[:], stats2_sb[:, 0:1], inv_n)
    nc.vector.tensor_scalar_mul(ex2_t[:], stats2_sb[:, 1:2], inv_n)
    nc.vector.tensor_tensor(out=tmp[:], in0=mean_t[:], in1=mean_t[:],
                            op=mybir.AluOpType.mult)
    nc.vector.tensor_tensor(out=var_t[:], in0=ex2_t[:], in1=tmp[:],
                            op=mybir.AluOpType.subtract)
    nc.vector.tensor_scalar_add(var_t[:], var_t[:], EPS)
    nc.scalar.activation(std_t[:], var_t[:],
                         mybir.ActivationFunctionType.Sqrt)
    nc.vector.reciprocal(rstd_t[:], std_t[:])
    nc.vector.tensor_tensor(out=s_vec[:], in0=rstd_t[:], in1=gamma_sb[:],
                            op=mybir.AluOpType.mult)
    nc.vector.tensor_tensor(out=tmp[:], in0=mean_t[:], in1=s_vec[:],
                            op=mybir.AluOpType.mult)
    nc.vector.tensor_tensor(out=t_vec[:], in0=beta_sb[:], in1=tmp[:],
                            op=mybir.AluOpType.subtract)
    return s_vec, t_vec


def _emit_phase2(nc, workp, psumtp, identity, out_t, out_ap, cfg, s_vec, t_vec):
    """BN apply + LeakyReLU + transpose back + writeback."""
    import concourse.mybir as mybir
    from concourse.bass import ts
    F32 = mybir.dt.float32
    ST, NB = cfg.st, cfg.st // 128

    for s in range(cfg.n_st):
        bn = workp.tile([128, ST], F32)
        nc.scalar.activation(bn[:], out_t[:, s * ST:(s + 1) * ST],
                             mybir.ActivationFunctionType.Identity,
                             bias=t_vec[:, 0:1], scale=s_vec[:, 0:1])
        bn2 = workp.tile([128, ST], F32)
        nc.vector.tensor_scalar_mul(bn2[:], bn[:], LEAK)
        nc.vector.tensor_tensor(out=bn[:], in0=bn[:], in1=bn2[:],
                                op=mybir.AluOpType.max)
        pt = psumtp.tile([128, ST], F32)
        for b in range(NB):
            nc.tensor.transpose(pt[:, ts(b, 128)], bn[:, ts(b, 128)],
                                identity[:])
        stage = workp.tile([128, ST], F32)
        nc.vector.tensor_copy(stage[:], pt[:])
        nc.sync.dma_start(
            out_ap[s * ST:(s + 1) * ST, :].rearrange("(b p) c -> p b c", p=128),
            stage[:].rearrange("p (b c) -> p b c", b=NB),
        )


def emit_sparse(tc, out_ap, ins, cfg, plans):
    """Valid-pair sparse conv kernel."""
    import concourse.mybir as mybir
    from concourse.bass import ts
    from concourse.masks import make_identity

    nc = tc.nc
    F32 = mybir.dt.float32
    BF16 = mybir.dt.bfloat16
    I16 = mybir.dt.int16
    ST, N_ST = cfg.st, cfg.n_st

    table, table_t, idx, slot = ins["table"], ins["table_t"], ins["idx"], ins["slot"]
    iota, w, gamma, beta = ins["iota"], ins["w"], ins["gamma"], ins["beta"]

    with (
        tc.tile_pool(name="const", bufs=1) as constp,
        tc.tile_pool(name="dram", bufs=1, space="DRAM") as dramp,
    ):
        table_sb = constp.tile([128, cfg.ranks * C], BF16)
        nc.sync.dma_start(table_sb[:].rearrange("p (r c) -> p r c", r=cfg.ranks),
                          table.rearrange("(r p) c -> p r c", p=128))
        w_sb = constp.tile([128, K * C], BF16)
        nc.sync.dma_start(w_sb[:].rearrange("ci (k co) -> ci k co", k=K),
                          w.rearrange("k ci co -> ci k co"))
        gamma_sb = constp.tile([128, 1], F32)
        nc.sync.dma_start(gamma_sb[:], gamma[:, None])
        beta_sb = constp.tile([128, 1], F32)
        nc.sync.dma_start(beta_sb[:], beta[:, None])
        iota_sb = constp.tile([128, ST], F32)
        nc.sync.dma_start(iota_sb[:], iota)
        identity = constp.tile([128, 128], F32)
        make_identity(nc, identity[:])
        identity_bf = constp.tile([128, 128], BF16)
        nc.vector.tensor_copy(identity_bf[:], identity[:])

        out_t = constp.tile([128, cfg.shard], BF16)   # staged pre-BN, transposed
        sum_part = constp.tile([128, N_ST], F32)
        sq_part = constp.tile([128, N_ST], F32)

        # ---- phase 1: conv + stats ----
        with (
            tc.tile_pool(name="gath", bufs=2) as gathp,
            tc.tile_pool(name="selfp", bufs=2) as selfp,
            tc.tile_pool(name="idxp", bufs=2) as idxp,
            tc.tile_pool(name="slotp", bufs=2) as slotp,
            tc.tile_pool(name="ztp", bufs=2) as ztp,
            tc.tile_pool(name="indp", bufs=3) as indp,
            tc.tile_pool(name="work1", bufs=2) as workp1,
            tc.tile_pool(name="ztps", bufs=1, space="PSUM") as ztpsump,
            tc.tile_pool(name="ztpm", bufs=1, space="PSUM") as ztpmp,
            tc.tile_pool(name="psum", bufs=2, space="PSUM") as psump,
        ):
            idx_off = 0   # int16 columns consumed
            slot_off = 0  # chunk columns consumed
            for s in range(N_ST):
                pl = plans[s]
                icols = pl.phat // 16
                it = idxp.tile([128, icols], I16)
                nc.sync.dma_start(it[:], idx[:, idx_off:idx_off + icols])
                sl = slotp.tile([128, pl.n_chunks], F32)
                nc.sync.dma_start(sl[:], slot[:, slot_off:slot_off + pl.n_chunks])
                idx_off += icols
                slot_off += pl.n_chunks

                gt = gathp.tile([128, 1, pl.phat], BF16)
                o = 0
                for csz in pl.gchunks:
                    nc.gpsimd.dma_gather(
                        gt[:, :, o:o + csz], table_sb[:],
                        it[:, o // 16:(o + csz) // 16], csz, csz, C,
                        transpose=True,
                        single_packet=True,
                        sbuf_tokens_per_rank=128,
                        sbuf_free_dim_per_rank=C * 2,
                    )
                    o += csz
                st_self = selfp.tile([128, ST], BF16)
                nc.sync.dma_start(st_self[:], table_t[:, s * ST:(s + 1) * ST])

                # step 1: ZtT[:, p] = W[k(p)]^T x[j(p)]  (channel-major)
                npair = pl.n_chunks * 128
                # fixed 2048-col PSUM tiles keep bank alignment
                zt_t_ps = ztpsump.tile([128, 128 * MAX_PAIR_CHUNKS], F32)
                for k, p0, p1 in pl.pieces:
                    nc.tensor.matmul(zt_t_ps[:, p0:p1],
                                     w_sb[:, ts(k, C)], gt[:, 0, p0:p1],
                                     start=True, stop=True)
                zt_t_sb = ztp.tile([128, npair], BF16)
                nc.scalar.activation(zt_t_sb[:], zt_t_ps[:, :npair],
                                     mybir.ActivationFunctionType.Identity)
                # transpose to pair-major Zt[p, :] for use as step-2 lhsT
                zt_pm_ps = ztpmp.tile([128, 128 * MAX_PAIR_CHUNKS], BF16)
                for cc in range(pl.n_chunks):
                    nc.tensor.transpose(zt_pm_ps[:, ts(cc, 128)],
                                        zt_t_sb[:, ts(cc, 128)], identity_bf[:])
                zt_sb = ztp.tile([128, npair], BF16)
                nc.scalar.activation(zt_sb[:], zt_pm_ps[:, :npair],
                                     mybir.ActivationFunctionType.Identity)

                # step 2: ps[:, slot] = W[13]^T x_self + sum_p Zt[p,:] Ind[p,slot]
                ps = psump.tile([128, ST], F32)
                nc.tensor.matmul(ps[:], w_sb[:, ts(SELF_K, C)], st_self[:],
                                 start=True, stop=False)
                for cc in range(pl.n_chunks):
                    ind = indp.tile([128, ST], BF16)
                    nc.vector.tensor_scalar(ind[:], iota_sb[:], sl[:, cc:cc + 1],
                                            None, mybir.AluOpType.is_equal)
                    nc.tensor.matmul(ps[:], zt_sb[:, ts(cc, 128)], ind[:],
                                     start=False, stop=(cc == pl.n_chunks - 1))

                nc.vector.reduce_sum(out=sum_part[:, s:s + 1], in_=ps[:],
                                     axis=mybir.AxisListType.X)
                trash = workp1.tile([128, ST], F32)
                nc.scalar.activation(trash[:], ps[:],
                                     mybir.ActivationFunctionType.Square,
                                     accum_out=sq_part[:, s:s + 1])
                nc.vector.tensor_copy(out_t[:, s * ST:(s + 1) * ST], ps[:])

        # ---- stats finalize + all-reduce + phase 2 ----
        stats_sb = constp.tile([128, 2], F32)
        nc.vector.reduce_sum(out=stats_sb[:, 0:1], in_=sum_part[:],
                             axis=mybir.AxisListType.X)
        nc.vector.reduce_sum(out=stats_sb[:, 1:2], in_=sq_part[:],
                             axis=mybir.AxisListType.X)
        s_vec, t_vec = _emit_bn_finalize(nc, tc, constp, dramp, cfg,
                                         stats_sb, gamma_sb, beta_sb)
        with (
            tc.tile_pool(name="work2", bufs=2) as workp2,
            tc.tile_pool(name="psumt", bufs=2, space="PSUM") as psumtp,
        ):
            _emit_phase2(nc, workp2, psumtp, identity, out_t, out_ap, cfg,
                         s_vec, t_vec)


def emit_dense(tc, out_ap, ins, cfg):
    """Dense 26-offset merged-gather kernel (fallback path)."""
    import concourse.mybir as mybir
    from concourse.bass import ts
    from concourse.masks import make_identity

    nc = tc.nc
    F32 = mybir.dt.float32
    BF16 = mybir.dt.bfloat16
    I16 = mybir.dt.int16
    ST, N_ST = cfg.st, cfg.n_st

    table, table_t, idx, w = ins["table"], ins["table_t"], ins["idx"], ins["w"]
    gamma, beta = ins["gamma"], ins["beta"]

    with (
        tc.tile_pool(name="const", bufs=1) as constp,
        tc.tile_pool(name="gath", bufs=2) as gathp,
        tc.tile_pool(name="selfp", bufs=2) as selfp,
        tc.tile_pool(name="idxp", bufs=2) as idxp,
        tc.tile_pool(name="work", bufs=2) as workp,
        tc.tile_pool(name="psum", bufs=2, space="PSUM") as psump,
        tc.tile_pool(name="psumt", bufs=2, space="PSUM") as psumtp,
        tc.tile_pool(name="dram", bufs=1, space="DRAM") as dramp,
    ):
        table_sb = constp.tile([128, cfg.ranks * C], BF16)
        nc.sync.dma_start(table_sb[:].rearrange("p (r c) -> p r c", r=cfg.ranks),
                          table.rearrange("(r p) c -> p r c", p=128))
        w_sb = constp.tile([128, K * C], BF16)
        nc.sync.dma_start(w_sb[:].rearrange("ci (k co) -> ci k co", k=K),
                          w.rearrange("k ci co -> ci k co"))
        gamma_sb = constp.tile([128, 1], F32)
        nc.sync.dma_start(gamma_sb[:], gamma[:, None])
        beta_sb = constp.tile([128, 1], F32)
        nc.sync.dma_start(beta_sb[:], beta[:, None])
        identity = constp.tile([128, 128], F32)
        make_identity(nc, identity[:])

        out_t = constp.tile([128, cfg.shard], BF16)
        sum_part = constp.tile([128, N_ST], F32)
        sq_part = constp.tile([128, N_ST], F32)

        for s in range(N_ST):
            it = idxp.tile([128, cfg.idx_cols], I16)
            nc.sync.dma_start(it[:], idx[:, s * cfg.idx_cols:(s + 1) * cfg.idx_cols])
            chunk = cfg.gather_chunk
            gt = gathp.tile([128, 1, cfg.merged], BF16)
            assert chunk and cfg.merged % chunk == 0
            cw = chunk // 16
            for gc in range(cfg.merged // chunk):
                nc.gpsimd.dma_gather(
                    gt[:, :, gc * chunk:(gc + 1) * chunk], table_sb[:],
                    it[:, gc * cw:(gc + 1) * cw], chunk, chunk, C,
                    transpose=True,
                    single_packet=True,
                    sbuf_tokens_per_rank=128,
                    sbuf_free_dim_per_rank=C * 2,
                )
            st_self = selfp.tile([128, ST], BF16)
            nc.sync.dma_start(st_self[:], table_t[:, s * ST:(s + 1) * ST])

            ps = psump.tile([128, ST], F32)
            for kk in range(K):
                if kk == SELF_K:
                    rhs = st_self[:]
                else:
                    kidx = KG_LIST.index(kk)
                    rhs = gt[:, 0, ts(kidx, ST)]
                nc.tensor.matmul(ps[:], w_sb[:, ts(kk, C)], rhs,
                                 start=(kk == 0), stop=(kk == K - 1))

            nc.vector.reduce_sum(out=sum_part[:, s:s + 1], in_=ps[:],
                                 axis=mybir.AxisListType.X)
            trash = workp.tile([128, ST], F32)
            nc.scalar.activation(trash[:], ps[:],
                                 mybir.ActivationFunctionType.Square,
                                 accum_out=sq_part[:, s:s + 1])
            nc.vector.tensor_copy(out_t[:, s * ST:(s + 1) * ST], ps[:])

        stats_sb = constp.tile([128, 2], F32)
        nc.vector.reduce_sum(out=stats_sb[:, 0:1], in_=sum_part[:],
                             axis=mybir.AxisListType.X)
        nc.vector.reduce_sum(out=stats_sb[:, 1:2], in_=sq_part[:],
                             axis=mybir.AxisListType.X)
        s_vec, t_vec = _emit_bn_finalize(nc, tc, constp, dramp, cfg,
                                         stats_sb, gamma_sb, beta_sb)
        _emit_phase2(nc, workp, psumtp, identity, out_t, out_ap, cfg,
                     s_vec, t_vec)


# ----------------------------------------------------------------------------
# host-side preparation
# ----------------------------------------------------------------------------

def _partition_components(nb, n, n_cores, shard_cap):
    """Whole-component LPT partition. Returns (members_per_core, ok)."""
    import scipy.sparse as sp
    import scipy.sparse.csgraph as csg
    import heapq

    valid = nb >= 0
    ii, kk = np.nonzero(valid)
    jj = nb[ii, kk]
    m = kk != SELF_K
    g = sp.coo_matrix((np.ones(m.sum(), np.int8), (ii[m], jj[m])), shape=(n, n))
    _, labels = csg.connected_components(g, directed=False)
    sizes = np.bincount(labels)
    if sizes.max() > shard_cap:
        return None, False
    order = np.argsort(sizes)[::-1]
    heap = [(0, c) for c in range(n_cores)]
    heapq.heapify(heap)
    assign = np.empty(len(sizes), np.int32)
    for comp in order:
        load, c = heapq.heappop(heap)
        assign[comp] = c
        heapq.heappush(heap, (load + int(sizes[comp]), c))
    if max(l for l, _ in heap) > shard_cap:
        return None, False
    shard_of = assign[labels]
    members = [np.nonzero(shard_of == c)[0] for c in range(n_cores)]
    return members, True


def _wrap_idx(m):
    """SWDGE idx layout: wrap a 1-D int16 list into 16 partitions, rep x8."""
    cols = len(m) // 16
    wrapped = m.reshape(cols, 16).T
    return np.tile(wrapped, (8, 1))


def _prepare_core_inputs_sparse(features, nb, members, cfg, plans, widths):
    """Per-core tables + compact k-major pair idx/slot arrays."""
    n = features.shape[0]
    loc = np.full(n, cfg.zero_row, np.int32)
    for mem in members:
        loc[mem] = np.arange(len(mem), dtype=np.int32)

    tot_icols = sum(pl.phat for pl in plans) // 16
    tot_chunks = sum(pl.n_chunks for pl in plans)

    in_maps = []
    for mem in members:
        real = len(mem)
        assert real <= cfg.shard
        table = np.zeros((cfg.table_rows, C), ml_dtypes.bfloat16)
        table[:real] = features[mem].astype(ml_dtypes.bfloat16)
        table_t = np.ascontiguousarray(table[:cfg.shard].T)  # [C, shard] bf16

        nb_c = nb[mem][:, KG_LIST]                  # [real, 26]
        v = nb_c >= 0
        li = loc[np.where(v, nb_c, 0)]
        assert (li[v] < real).all(), "neighbor escaped shard"

        idx_dram = np.empty((128, tot_icols), np.int16)
        slot_dram = np.empty((128, tot_chunks), np.float32)
        io = 0
        so = 0
        for s, pl in enumerate(plans):
            r0 = s * cfg.st
            r1 = min((s + 1) * cfg.st, real)
            w = widths[s]
            jl = np.full(pl.phat, cfg.zero_row, np.int64)
            sl = np.full(pl.phat, -1.0, np.float64)
            p = 0
            if r1 > r0:
                vt = v[r0:r1]
                lt = li[r0:r1]
                for ki in range(cfg.kg):
                    slots = np.nonzero(vt[:, ki])[0]
                    jl[p:p + len(slots)] = lt[slots, ki]
                    sl[p:p + len(slots)] = slots
                    p += int(w[ki])
                assert p <= pl.phat
            icols = pl.phat // 16
            idx_dram[:, io:io + icols] = _wrap_idx(jl.astype(np.int16))
            slot_dram[:, so:so + pl.n_chunks] = \
                sl.astype(np.float32).reshape(pl.n_chunks, 128).T
            io += icols
            so += pl.n_chunks
        assert io == tot_icols and so == tot_chunks
        in_maps.append({"table": table, "table_t": table_t,
                        "idx": idx_dram, "slot": slot_dram})
    return in_maps


def _prepare_core_inputs_dense(features, nb, members, cfg):
    n = features.shape[0]
    loc = np.full(n, cfg.zero_row, np.int32)
    for mem in members:
        loc[mem] = np.arange(len(mem), dtype=np.int32)

    in_maps = []
    for mem in members:
        real = len(mem)
        assert real <= cfg.shard
        table = np.zeros((cfg.table_rows, C), ml_dtypes.bfloat16)
        table[:real] = features[mem].astype(ml_dtypes.bfloat16)
        table_t = np.ascontiguousarray(table[:cfg.shard].T)

        idx16 = np.full((cfg.shard, cfg.kg), cfg.zero_row, np.int32)
        nb_c = nb[mem][:, KG_LIST]
        v = nb_c >= 0
        li = loc[np.where(v, nb_c, 0)]
        assert (li[v] < real).all(), "neighbor escaped shard"
        idx16[:real] = np.where(v, li, cfg.zero_row)
        idx16 = idx16.astype(np.int16)

        idx_dram = np.empty((128, cfg.n_st * cfg.idx_cols), np.int16)
        for s in range(cfg.n_st):
            m = idx16[s * cfg.st:(s + 1) * cfg.st, :].T.reshape(-1)  # k-major
            idx_dram[:, s * cfg.idx_cols:(s + 1) * cfg.idx_cols] = _wrap_idx(m)
        in_maps.append({"table": table, "table_t": table_t, "idx": idx_dram})
    return in_maps


def _reference_fallback(features, w, b, gamma, beta, nb):
    feats = np.asarray(features, np.float32)
    wf = np.asarray(w, np.float32)
    out = np.broadcast_to(np.asarray(b, np.float32), feats.shape).copy()
    valid = nb >= 0
    idx = np.where(valid, nb, 0)
    for k in range(K):
        xk = feats[idx[:, k]] * valid[:, k:k + 1]
        out += xk @ wf[k]
    mean = out.mean(0)
    var = out.var(0)
    out = (out - mean) / np.sqrt(var + EPS) * np.asarray(gamma, np.float32) \
        + np.asarray(beta, np.float32)
    return np.where(out > 0, out, LEAK * out).astype(np.float32)


def _build_bass(cfg, reps=1, plans=None):
    import concourse.bacc as bacc
    import concourse.mybir as mybir
    import concourse.tile as tile

    nc = bacc.Bacc("TRN2", target_bir_lowering=False, debug=False,
                   num_devices=cfg.n_cores)
    F32 = mybir.dt.float32
    BF16 = mybir.dt.bfloat16
    I16 = mybir.dt.int16
    ins = {
        "table": nc.dram_tensor("table", [cfg.table_rows, C], BF16,
                                kind="ExternalInput")[:, :],
        "table_t": nc.dram_tensor("table_t", [C, cfg.shard], BF16,
                                  kind="ExternalInput")[:, :],
        "w": nc.dram_tensor("w", [K, C, C], BF16, kind="ExternalInput")[:, :, :],
        "gamma": nc.dram_tensor("gamma", [C], F32, kind="ExternalInput")[:],
        "beta": nc.dram_tensor("beta", [C], F32, kind="ExternalInput")[:],
    }
    if plans is None:
        ins["idx"] = nc.dram_tensor(
            "idx", [128, cfg.n_st * cfg.idx_cols], I16, kind="ExternalInput")[:, :]
    else:
        tot_icols = sum(pl.phat for pl in plans) // 16
        tot_chunks = sum(pl.n_chunks for pl in plans)
        ins["idx"] = nc.dram_tensor(
            "idx", [128, tot_icols], I16, kind="ExternalInput")[:, :]
        ins["slot"] = nc.dram_tensor(
            "slot", [128, tot_chunks], F32, kind="ExternalInput")[:, :]
        ins["iota"] = nc.dram_tensor(
            "iota", [128, cfg.st], F32, kind="ExternalInput")[:, :]
    out = nc.dram_tensor("out", [cfg.shard, C], F32, kind="ExternalOutput")
    with tile.TileContext(nc) as tc:
        for _ in range(reps):
            if plans is None:
                emit_dense(tc, out[:, :], ins, cfg)
            else:
                emit_sparse(tc, out[:, :], ins, cfg, plans)
    nc.compile()
    return nc


def prepare_all(features, W, nb, cfg):
    """Partition + plan + per-core input maps. Returns (core_maps, members,
    plans) with plans=None meaning the dense fallback program."""
    members, ok = _partition_components(nb, cfg.n_total, cfg.n_cores, cfg.shard)
    if not ok:
        return None, None, None
    plans, widths = build_plan(nb, members, cfg)
    if plans is not None:
        core_maps = _prepare_core_inputs_sparse(features, nb, members, cfg,
                                                plans, widths)
        iota = np.tile(np.arange(cfg.st, dtype=np.float32), (128, 1))
        extra = {"iota": iota}
    else:
        core_maps = _prepare_core_inputs_dense(features, nb, members, cfg)
        extra = {}
    w_bf = np.asarray(W, np.float32).astype(ml_dtypes.bfloat16)
    for m in core_maps:
        m["w"] = w_bf
        m.update(extra)
    return core_maps, members, plans


def kernel(features, W, b, gamma, beta, neighbor_idx):
    from concourse.bass_utils import run_bass_kernel_spmd

    features = np.asarray(features, np.float32)
    Wf = np.asarray(W, np.float32)
    gamma_f = np.asarray(gamma, np.float32)
    beta_f = np.asarray(beta, np.float32)
    nb = np.asarray(neighbor_idx, np.int32)
    cfg = FULL_CFG
    assert features.shape == (cfg.n_total, C)

    core_maps, members, plans = prepare_all(features, Wf, nb, cfg)
    if core_maps is None:
        return _reference_fallback(features, Wf, b, gamma_f, beta_f, nb)
    for m in core_maps:
        m["gamma"] = gamma_f
        m["beta"] = beta_f

    nc = _build_bass(cfg, plans=plans)
    res = run_bass_kernel_spmd(nc, core_maps, core_ids=list(range(cfg.n_cores)))

    out_full = np.empty((cfg.n_total, C), np.float32)
    for c, mem in enumerate(members):
        out_full[mem] = res.results[c]["out"][:len(mem)]
    return out_full


# revision 15
# speedup vs baseline: 22.0511x; 3.7796x over previous
"""Submanifold 3x3x3 sparse conv (gnn_message_passing) + BatchNorm + LeakyReLU
on 8 Trainium2 NeuronCores.

Strategy (hardcoded for N=200000, C=128, K=27, GRID=128^3 @ ~9.5% occupancy):
  * The active-voxel neighbor graph at this occupancy splits into ~31k tiny
    connected components (max ~2.4k voxels). Whole components are partitioned
    across the 8 cores (LPT bin packing) -> every neighbor reference stays
    inside its core's shard; shard-local indices fit in int16, required by
    the SWDGE dma_gather ucode.
  * At 9.5% occupancy only ~2.44 of the 26 non-self neighbor slots are
    active per voxel.  Instead of gathering all 26 rows per voxel (the
    SWDGE-bound dense scheme), each 512-row supertile gathers ONLY the
    valid (slot, k) pairs, k-major, into a compact [128, P] bf16 tile
    (~1.5k tokens instead of 13.3k).  Per-(tile,k) group widths are maxed
    across the 8 cores so one SPMD program serves all shards; shards pad
    their groups with zero-row tokens.
  * Step 1: per-k-range matmuls (lhsT = gathered columns, rhs = W[k])
    produce Zt [pairs, C_out] in PSUM, copied to SBUF as bf16 by ACT.
  * Step 2: the scatter-accumulate out[:, slot] += Zt[p, :] for
    slot = slot_of[p] is a matmul against a 0/1 indicator matrix
    Ind[p, slot] = (slot_of[p] == slot), generated on DVE per 128-pair
    chunk via tensor_scalar is_equal against a host-uploaded iota row.
    The self offset (k=13) is one dense matmul from a host-pretransposed
    table slice.  All accumulate into fp32 PSUM [C_out, 512].
  * Gathers are issued in <=256-idx single-packet chunks: SWDGE cost is
    ~0.7us per instruction nearly independent of idx count up to 256
    (512-idx chunks hit a ~6us/instruction pathology; HW-measured
    2026-08-08).
  * BN statistics: per-supertile DVE reduce (sum) + ACT Square with
    accum_out (sum of squares), finalized and all-reduced across the 8
    cores with one tiny AllReduce collective. b is ignored: BatchNorm is
    shift-invariant so the conv bias cancels exactly.
  * BN apply + LeakyReLU per tile, then PE transposes back to row-major
    and contiguous DMA writeback. Host reassembles shards and inverts the
    component permutation.

Falls back to the dense-gather kernel if the valid-pair structure exceeds
the sparse path's PSUM budget, and to pure numpy if the graph is not
separable into <=25088-row shards (never the case for the intended input
distribution).
"""

import numpy as np
import ml_dtypes

C = 128
K = 27
EPS = 1e-4
LEAK = 0.333
N_CORES = 8
SELF_K = 13
KG_LIST = [k for k in range(K) if k != SELF_K]


class Cfg:
    def __init__(self, n_total, st, n_st, table_rows, n_cores):
        assert st % 128 == 0 and table_rows % 128 == 0
        self.n_total = n_total          # global number of real rows (stats divisor)
        self.st = st                    # supertile rows
        self.n_st = n_st                # supertiles per core
        self.shard = st * n_st          # padded rows per core
        self.table_rows = table_rows    # shard table rows incl. zero pad
        self.ranks = table_rows // 128
        self.zero_row = table_rows - 1
        self.n_cores = n_cores
        self.kg = K - 1                 # gathered (non-self) offsets
        self.merged = self.kg * st      # idxs per merged gather (dense path)
        self.idx_cols = self.merged // 16
        assert self.merged % 128 == 0
        # SWDGE gathers: ~0.7us fixed cost per instruction for <=256 idxs;
        # 512-idx single-packet chunks are ~9x slower per idx (HW-measured
        # 2026-08-08). Keep chunks at 256.
        self.gather_chunk = 256 if self.merged % 256 == 0 else (
            128 if self.merged % 128 == 0 else 0)


FULL_CFG = Cfg(n_total=200_000, st=512, n_st=49, table_rows=25_216, n_cores=N_CORES)

MAX_PAIR_CHUNKS = 16   # Zt PSUM budget: 16 chunks * 512B = 4 banks


class TilePlan:
    """Per-supertile static structure, common to all cores (SPMD)."""
    __slots__ = ("phat", "n_chunks", "gchunks", "pieces")

    def __init__(self, phat, gchunks, pieces):
        self.phat = phat                  # padded pair count (x128)
        self.n_chunks = phat // 128       # 128-pair Ind/Zt chunks
        self.gchunks = gchunks            # gather chunk sizes (<=256, x128)
        self.pieces = pieces              # [(k, p0, p1)] matmul pieces


def build_plan(nb, members, cfg):
    """Common (max-over-cores) per-tile pair structure. Returns
    (plans, widths) where widths[s] = per-k group widths."""
    plans, widths = [], []
    valid = [nb[mem][:, KG_LIST] >= 0 for mem in members]  # [real, 26] each
    for s in range(cfg.n_st):
        r0, r1 = s * cfg.st, (s + 1) * cfg.st
        w = np.zeros(cfg.kg, np.int64)
        for v in valid:
            tile = v[r0:r1]
            if tile.shape[0]:
                np.maximum(w, tile.sum(0), out=w)
        total = int(w.sum())
        phat = max(128, -(-total // 128) * 128)
        if phat // 128 > MAX_PAIR_CHUNKS:
            return None, None
        w[-1] += phat - total  # pad tail into last group
        n128 = phat // 128
        gchunks = [256] * (n128 // 2) + [128] * (n128 % 2)
        # matmul pieces: ZtT free-axis ranges, split at 512-col PSUM banks
        pieces = []
        p = 0
        for ki, wk in enumerate(w):
            k = KG_LIST[ki]
            left = int(wk)
            while left:
                take = min(left, 512 - (p % 512))
                pieces.append((k, p, p + take))
                p += take
                left -= take
        assert p == phat
        plans.append(TilePlan(phat, gchunks, pieces))
        widths.append(w)
    return plans, widths


def _emit_bn_finalize(nc, tc, constp, dramp, cfg, stats_sb, gamma_sb, beta_sb):
    """All-reduce raw sums, produce per-channel scale/shift vectors."""
    import concourse.mybir as mybir
    F32 = mybir.dt.float32

    if cfg.n_cores > 1:
        stats_in = dramp.tile([128, 2], F32)
        stats_out = dramp.tile([128, 2], F32)
        nc.sync.dma_start(stats_in[:], stats_sb[:])
        nc.gpsimd.collective_compute(
            "AllReduce", mybir.AluOpType.add,
            replica_groups=[list(range(cfg.n_cores))],
            ins=[stats_in.opt()], outs=[stats_out.opt()],
        )
        stats2_sb = constp.tile([128, 2], F32)
        nc.sync.dma_start(stats2_sb[:], stats_out[:])
    else:
        stats2_sb = stats_sb

    mean_t = constp.tile([128, 1], F32)
    ex2_t = constp.tile([128, 1], F32)
    var_t = constp.tile([128, 1], F32)
    std_t = constp.tile([128, 1], F32)
    rstd_t = constp.tile([128, 1], F32)
    s_vec = constp.tile([128, 1], F32)
    t_vec = constp.tile([128, 1], F32)
    tmp = constp.tile([128, 1], F32)
    inv_n = 1.0 / cfg.n_total
    nc.vector.tensor_scalar_mul(mean_t[:], stats2_sb[:, 0:1], inv_n)
    nc.vector.tensor_scalar_mul(ex2_t[:], stats2_sb[:, 1:2], inv_n)
    nc.vector.tensor_tensor(out=tmp[:], in0=mean_t[:], in1=mean_t[:],
                            op=mybir.AluOpType.mult)
    nc.vector.tensor_tensor(out=var_t[:], in0=ex2_t[:], in1=tmp[:],
                            op=mybir.AluOpType.subtract)
    nc.vector.tensor_scalar_add(var_t[:], var_t[:], EPS)
    nc.scalar.activation(std_t[:], var_t[:],
                         mybir.ActivationFunctionType.Sqrt)
    nc.vector.reciprocal(rstd_t[:], std_t[:])
    nc.vector.tensor_tensor(out=s_vec[:], in0=rstd_t[:], in1=gamma_sb[:],
                            op=mybir.AluOpType.mult)
    nc.vector.tensor_tensor(out=tmp[:], in0=mean_t[:], in1=s_vec[:],
                            op=mybir.AluOpType.mult)
    nc.vector.tensor_tensor(out=t_vec[:], in0=beta_sb[:], in1=tmp[:],
                            op=mybir.AluOpType.subtract)
    return s_vec, t_vec


def _emit_phase2(nc, workp, psumtp, identity, out_t, out_ap, cfg, s_vec, t_vec):
    """BN apply + LeakyReLU + transpose back + writeback."""
    import concourse.mybir as mybir
    from concourse.bass import ts
    F32 = mybir.dt.float32
    ST, NB = cfg.st, cfg.st // 128

    for s in range(cfg.n_st):
        # out = lrelu(x*scale + shift) fused on ACT
        bn = workp.tile([128, ST], F32)
        nc.scalar.activation(bn[:], out_t[:, s * ST:(s + 1) * ST],
                             mybir.ActivationFunctionType.Lrelu,
                             bias=t_vec[:, 0:1], scale=s_vec[:, 0:1],
                             alpha=LEAK)
        pt = psumtp.tile([128, ST], F32)
        for b in range(NB):
            nc.tensor.transpose(pt[:, ts(b, 128)], bn[:, ts(b, 128)],
                                identity[:])
        stage = workp.tile([128, ST], F32)
        nc.vector.tensor_copy(stage[:], pt[:])
        nc.sync.dma_start(
            out_ap[s * ST:(s + 1) * ST, :].rearrange("(b p) c -> p b c", p=128),
            stage[:].rearrange("p (b c) -> p b c", b=NB),
        )


def emit_sparse(tc, out_ap, ins, cfg, plans):
    """Valid-pair sparse conv kernel."""
    import concourse.mybir as mybir
    from concourse.bass import ts
    from concourse.masks import make_identity

    nc = tc.nc
    F32 = mybir.dt.float32
    BF16 = mybir.dt.bfloat16
    I16 = mybir.dt.int16
    ST, N_ST = cfg.st, cfg.n_st

    table, table_t, idx, slot = ins["table"], ins["table_t"], ins["idx"], ins["slot"]
    iota, w, gamma, beta = ins["iota"], ins["w"], ins["gamma"], ins["beta"]

    with (
        tc.tile_pool(name="const", bufs=1) as constp,
        tc.tile_pool(name="dram", bufs=1, space="DRAM") as dramp,
    ):
        table_sb = constp.tile([128, cfg.ranks * C], BF16)
        nc.sync.dma_start(table_sb[:].rearrange("p (r c) -> p r c", r=cfg.ranks),
                          table.rearrange("(r p) c -> p r c", p=128))
        w_sb = constp.tile([128, K * C], BF16)
        nc.sync.dma_start(w_sb[:].rearrange("ci (k co) -> ci k co", k=K),
                          w.rearrange("k ci co -> ci k co"))
        gamma_sb = constp.tile([128, 1], F32)
        nc.sync.dma_start(gamma_sb[:], gamma[:, None])
        beta_sb = constp.tile([128, 1], F32)
        nc.sync.dma_start(beta_sb[:], beta[:, None])
        iota_sb = constp.tile([128, ST], F32)
        nc.sync.dma_start(iota_sb[:], iota)
        identity = constp.tile([128, 128], F32)
        make_identity(nc, identity[:])

        out_t = constp.tile([128, cfg.shard], BF16)   # staged pre-BN, transposed
        sum_part = constp.tile([128, N_ST], F32)
        sq_part = constp.tile([128, N_ST], F32)

        # ---- phase 1: conv + stats ----
        with (
            tc.tile_pool(name="gath", bufs=2) as gathp,
            tc.tile_pool(name="selfp", bufs=2) as selfp,
            tc.tile_pool(name="idxp", bufs=2) as idxp,
            tc.tile_pool(name="slotp", bufs=2) as slotp,
            tc.tile_pool(name="ztp", bufs=2) as ztp,
            tc.tile_pool(name="indp", bufs=3) as indp,
            tc.tile_pool(name="work1", bufs=2) as workp1,
            tc.tile_pool(name="ztps", bufs=1, space="PSUM") as ztpsump,
            tc.tile_pool(name="psum", bufs=2, space="PSUM") as psump,
        ):
            def emit_front(s):
                """Gather + step-1 transform + xbar transpose for tile s.
                Returns (zt_sb, st_self, sl, n_chunks)."""
                pl = plans[s]
                icols = pl.phat // 16
                it = idxp.tile([128, icols], I16)
                nc.sync.dma_start(
                    it[:], idx[:, offs[s][0]:offs[s][0] + icols])
                sl = slotp.tile([128, pl.n_chunks], F32)
                nc.sync.dma_start(
                    sl[:], slot[:, offs[s][1]:offs[s][1] + pl.n_chunks])

                gt = gathp.tile([128, 1, pl.phat], BF16)
                o = 0
                for csz in pl.gchunks:
                    nc.gpsimd.dma_gather(
                        gt[:, :, o:o + csz], table_sb[:],
                        it[:, o // 16:(o + csz) // 16], csz, csz, C,
                        transpose=True,
                        single_packet=True,
                        sbuf_tokens_per_rank=128,
                        sbuf_free_dim_per_rank=C * 2,
                    )
                    o += csz
                st_self = selfp.tile([128, ST], BF16)
                nc.sync.dma_start(st_self[:], table_t[:, s * ST:(s + 1) * ST])

                # step 1: ZtT[:, p] = W[k(p)]^T x[j(p)]  (channel-major)
                npair = pl.n_chunks * 128
                # fixed-size PSUM tile keeps bank alignment
                zt_t_ps = ztpsump.tile([128, 128 * MAX_PAIR_CHUNKS], F32)
                for k, p0, p1 in pl.pieces:
                    nc.tensor.matmul(zt_t_ps[:, p0:p1],
                                     w_sb[:, ts(k, C)], gt[:, 0, p0:p1],
                                     start=True, stop=True)
                zt_t_sb = ztp.tile([128, npair], BF16)
                nc.scalar.activation(zt_t_sb[:], zt_t_ps[:, :npair],
                                     mybir.ActivationFunctionType.Identity)
                # xbar transpose to pair-major Zt[p, :] for step-2 lhsT
                zt_sb = ztp.tile([128, npair], BF16)
                for cc in range(pl.n_chunks):
                    nc.sync.dma_start_transpose(zt_sb[:, ts(cc, 128)],
                                                zt_t_sb[:, ts(cc, 128)])
                return zt_sb, st_self, sl, pl.n_chunks

            def emit_back(s, st):
                """Step-2 scatter matmuls + stats for tile s."""
                zt_sb, st_self, sl, n_chunks = st
                ps = psump.tile([128, ST], F32)
                nc.tensor.matmul(ps[:], w_sb[:, ts(SELF_K, C)], st_self[:],
                                 start=True, stop=False)
                for cc in range(n_chunks):
                    ind = indp.tile([128, ST], BF16)
                    nc.vector.tensor_scalar(ind[:], iota_sb[:],
                                            sl[:, cc:cc + 1],
                                            None, mybir.AluOpType.is_equal)
                    nc.tensor.matmul(ps[:], zt_sb[:, ts(cc, 128)], ind[:],
                                     start=False, stop=(cc == n_chunks - 1))

                nc.vector.reduce_sum(out=sum_part[:, s:s + 1], in_=ps[:],
                                     axis=mybir.AxisListType.X)
                trash = workp1.tile([128, ST], F32)
                nc.scalar.activation(trash[:], ps[:],
                                     mybir.ActivationFunctionType.Square,
                                     accum_out=sq_part[:, s:s + 1])
                nc.vector.tensor_copy(out_t[:, s * ST:(s + 1) * ST], ps[:])

            offs = []
            io = so = 0
            for pl in plans:
                offs.append((io, so))
                io += pl.phat // 16
                so += pl.n_chunks
            # depth-1 software pipeline: PE runs step-2 of tile s-1 while
            # ACT copy + xbar transpose of tile s are in flight.
            pending = None
            for s in range(N_ST):
                st = emit_front(s)
                if pending is not None:
                    emit_back(s - 1, pending)
                pending = st
            emit_back(N_ST - 1, pending)

        # ---- stats finalize + all-reduce + phase 2 ----
        stats_sb = constp.tile([128, 2], F32)
        nc.vector.reduce_sum(out=stats_sb[:, 0:1], in_=sum_part[:],
                             axis=mybir.AxisListType.X)
        nc.vector.reduce_sum(out=stats_sb[:, 1:2], in_=sq_part[:],
                             axis=mybir.AxisListType.X)
        s_vec, t_vec = _emit_bn_finalize(nc, tc, constp, dramp, cfg,
                                         stats_sb, gamma_sb, beta_sb)
        with (
            tc.tile_pool(name="work2", bufs=2) as workp2,
            tc.tile_pool(name="psumt", bufs=2, space="PSUM") as psumtp,
        ):
            _emit_phase2(nc, workp2, psumtp, identity, out_t, out_ap, cfg,
                         s_vec, t_vec)


def emit_dense(tc, out_ap, ins, cfg):
    """Dense 26-offset merged-gather kernel (fallback path)."""
    import concourse.mybir as mybir
    from concourse.bass import ts
    from concourse.masks import make_identity

    nc = tc.nc
    F32 = mybir.dt.float32
    BF16 = mybir.dt.bfloat16
    I16 = mybir.dt.int16
    ST, N_ST = cfg.st, cfg.n_st

    table, table_t, idx, w = ins["table"], ins["table_t"], ins["idx"], ins["w"]
    gamma, beta = ins["gamma"], ins["beta"]

    with (
        tc.tile_pool(name="const", bufs=1) as constp,
        tc.tile_pool(name="gath", bufs=2) as gathp,
        tc.tile_pool(name="selfp", bufs=2) as selfp,
        tc.tile_pool(name="idxp", bufs=2) as idxp,
        tc.tile_pool(name="work", bufs=2) as workp,
        tc.tile_pool(name="psum", bufs=2, space="PSUM") as psump,
        tc.tile_pool(name="psumt", bufs=2, space="PSUM") as psumtp,
        tc.tile_pool(name="dram", bufs=1, space="DRAM") as dramp,
    ):
        table_sb = constp.tile([128, cfg.ranks * C], BF16)
        nc.sync.dma_start(table_sb[:].rearrange("p (r c) -> p r c", r=cfg.ranks),
                          table.rearrange("(r p) c -> p r c", p=128))
        w_sb = constp.tile([128, K * C], BF16)
        nc.sync.dma_start(w_sb[:].rearrange("ci (k co) -> ci k co", k=K),
                          w.rearrange("k ci co -> ci k co"))
        gamma_sb = constp.tile([128, 1], F32)
        nc.sync.dma_start(gamma_sb[:], gamma[:, None])
        beta_sb = constp.tile([128, 1], F32)
        nc.sync.dma_start(beta_sb[:], beta[:, None])
        identity = constp.tile([128, 128], F32)
        make_identity(nc, identity[:])

        out_t = constp.tile([128, cfg.shard], BF16)
        sum_part = constp.tile([128, N_ST], F32)
        sq_part = constp.tile([128, N_ST], F32)

        for s in range(N_ST):
            it = idxp.tile([128, cfg.idx_cols], I16)
            nc.sync.dma_start(it[:], idx[:, s * cfg.idx_cols:(s + 1) * cfg.idx_cols])
            chunk = cfg.gather_chunk
            gt = gathp.tile([128, 1, cfg.merged], BF16)
            assert chunk and cfg.merged % chunk == 0
            cw = chunk // 16
            for gc in range(cfg.merged // chunk):
                nc.gpsimd.dma_gather(
                    gt[:, :, gc * chunk:(gc + 1) * chunk], table_sb[:],
                    it[:, gc * cw:(gc + 1) * cw], chunk, chunk, C,
                    transpose=True,
                    single_packet=True,
                    sbuf_tokens_per_rank=128,
                    sbuf_free_dim_per_rank=C * 2,
                )
            st_self = selfp.tile([128, ST], BF16)
            nc.sync.dma_start(st_self[:], table_t[:, s * ST:(s + 1) * ST])

            ps = psump.tile([128, ST], F32)
            for kk in range(K):
                if kk == SELF_K:
                    rhs = st_self[:]
                else:
                    kidx = KG_LIST.index(kk)
                    rhs = gt[:, 0, ts(kidx, ST)]
                nc.tensor.matmul(ps[:], w_sb[:, ts(kk, C)], rhs,
                                 start=(kk == 0), stop=(kk == K - 1))

            nc.vector.reduce_sum(out=sum_part[:, s:s + 1], in_=ps[:],
                                 axis=mybir.AxisListType.X)
            trash = workp.tile([128, ST], F32)
            nc.scalar.activation(trash[:], ps[:],
                                 mybir.ActivationFunctionType.Square,
                                 accum_out=sq_part[:, s:s + 1])
            nc.vector.tensor_copy(out_t[:, s * ST:(s + 1) * ST], ps[:])

        stats_sb = constp.tile([128, 2], F32)
        nc.vector.reduce_sum(out=stats_sb[:, 0:1], in_=sum_part[:],
                             axis=mybir.AxisListType.X)
        nc.vector.reduce_sum(out=stats_sb[:, 1:2], in_=sq_part[:],
                             axis=mybir.AxisListType.X)
        s_vec, t_vec = _emit_bn_finalize(nc, tc, constp, dramp, cfg,
                                         stats_sb, gamma_sb, beta_sb)
        _emit_phase2(nc, workp, psumtp, identity, out_t, out_ap, cfg,
                     s_vec, t_vec)


# ----------------------------------------------------------------------------
# host-side preparation
# ----------------------------------------------------------------------------

def _partition_components(nb, n, n_cores, shard_cap):
    """Whole-component LPT partition. Returns (members_per_core, ok)."""
    import scipy.sparse as sp
    import scipy.sparse.csgraph as csg
    import heapq

    valid = nb >= 0
    ii, kk = np.nonzero(valid)
    jj = nb[ii, kk]
    m = kk != SELF_K
    g = sp.coo_matrix((np.ones(m.sum(), np.int8), (ii[m], jj[m])), shape=(n, n))
    _, labels = csg.connected_components(g, directed=False)
    sizes = np.bincount(labels)
    if sizes.max() > shard_cap:
        return None, False
    order = np.argsort(sizes)[::-1]
    heap = [(0, c) for c in range(n_cores)]
    heapq.heapify(heap)
    assign = np.empty(len(sizes), np.int32)
    for comp in order:
        load, c = heapq.heappop(heap)
        assign[comp] = c
        heapq.heappush(heap, (load + int(sizes[comp]), c))
    if max(l for l, _ in heap) > shard_cap:
        return None, False
    shard_of = assign[labels]
    members = [np.nonzero(shard_of == c)[0] for c in range(n_cores)]
    return members, True


def _wrap_idx(m):
    """SWDGE idx layout: wrap a 1-D int16 list into 16 partitions, rep x8."""
    cols = len(m) // 16
    wrapped = m.reshape(cols, 16).T
    return np.tile(wrapped, (8, 1))


def _prepare_core_inputs_sparse(features, nb, members, cfg, plans, widths):
    """Per-core tables + compact k-major pair idx/slot arrays."""
    n = features.shape[0]
    loc = np.full(n, cfg.zero_row, np.int32)
    for mem in members:
        loc[mem] = np.arange(len(mem), dtype=np.int32)

    tot_icols = sum(pl.phat for pl in plans) // 16
    tot_chunks = sum(pl.n_chunks for pl in plans)

    in_maps = []
    for mem in members:
        real = len(mem)
        assert real <= cfg.shard
        table = np.zeros((cfg.table_rows, C), ml_dtypes.bfloat16)
        table[:real] = features[mem].astype(ml_dtypes.bfloat16)
        table_t = np.ascontiguousarray(table[:cfg.shard].T)  # [C, shard] bf16

        nb_c = nb[mem][:, KG_LIST]                  # [real, 26]
        v = nb_c >= 0
        li = loc[np.where(v, nb_c, 0)]
        assert (li[v] < real).all(), "neighbor escaped shard"

        idx_dram = np.empty((128, tot_icols), np.int16)
        slot_dram = np.empty((128, tot_chunks), np.float32)
        io = 0
        so = 0
        for s, pl in enumerate(plans):
            r0 = s * cfg.st
            r1 = min((s + 1) * cfg.st, real)
            w = widths[s]
            jl = np.full(pl.phat, cfg.zero_row, np.int64)
            sl = np.full(pl.phat, -1.0, np.float64)
            p = 0
            if r1 > r0:
                vt = v[r0:r1]
                lt = li[r0:r1]
                for ki in range(cfg.kg):
                    slots = np.nonzero(vt[:, ki])[0]
                    jl[p:p + len(slots)] = lt[slots, ki]
                    sl[p:p + len(slots)] = slots
                    p += int(w[ki])
                assert p <= pl.phat
            icols = pl.phat // 16
            idx_dram[:, io:io + icols] = _wrap_idx(jl.astype(np.int16))
            slot_dram[:, so:so + pl.n_chunks] = \
                sl.astype(np.float32).reshape(pl.n_chunks, 128).T
            io += icols
            so += pl.n_chunks
        assert io == tot_icols and so == tot_chunks
        in_maps.append({"table": table, "table_t": table_t,
                        "idx": idx_dram, "slot": slot_dram})
    return in_maps


def _prepare_core_inputs_dense(features, nb, members, cfg):
    n = features.shape[0]
    loc = np.full(n, cfg.zero_row, np.int32)
    for mem in members:
        loc[mem] = np.arange(len(mem), dtype=np.int32)

    in_maps = []
    for mem in members:
        real = len(mem)
        assert real <= cfg.shard
        table = np.zeros((cfg.table_rows, C), ml_dtypes.bfloat16)
        table[:real] = features[mem].astype(ml_dtypes.bfloat16)
        table_t = np.ascontiguousarray(table[:cfg.shard].T)

        idx16 = np.full((cfg.shard, cfg.kg), cfg.zero_row, np.int32)
        nb_c = nb[mem][:, KG_LIST]
        v = nb_c >= 0
        li = loc[np.where(v, nb_c, 0)]
        assert (li[v] < real).all(), "neighbor escaped shard"
        idx16[:real] = np.where(v, li, cfg.zero_row)
        idx16 = idx16.astype(np.int16)

        idx_dram = np.empty((128, cfg.n_st * cfg.idx_cols), np.int16)
        for s in range(cfg.n_st):
            m = idx16[s * cfg.st:(s + 1) * cfg.st, :].T.reshape(-1)  # k-major
            idx_dram[:, s * cfg.idx_cols:(s + 1) * cfg.idx_cols] = _wrap_idx(m)
        in_maps.append({"table": table, "table_t": table_t, "idx": idx_dram})
    return in_maps


def _reference_fallback(features, w, b, gamma, beta, nb):
    feats = np.asarray(features, np.float32)
    wf = np.asarray(w, np.float32)
    out = np.broadcast_to(np.asarray(b, np.float32), feats.shape).copy()
    valid = nb >= 0
    idx = np.where(valid, nb, 0)
    for k in range(K):
        xk = feats[idx[:, k]] * valid[:, k:k + 1]
        out += xk @ wf[k]
    mean = out.mean(0)
    var = out.var(0)
    out = (out - mean) / np.sqrt(var + EPS) * np.asarray(gamma, np.float32) \
        + np.asarray(beta, np.float32)
    return np.where(out > 0, out, LEAK * out).astype(np.float32)


def _build_bass(cfg, reps=1, plans=None):
    import concourse.bacc as bacc
    import concourse.mybir as mybir
    import concourse.tile as tile

    nc = bacc.Bacc("TRN2", target_bir_lowering=False, debug=False,
                   num_devices=cfg.n_cores)
    F32 = mybir.dt.float32
    BF16 = mybir.dt.bfloat16
    I16 = mybir.dt.int16
    ins = {
        "table": nc.dram_tensor("table", [cfg.table_rows, C], BF16,
                                kind="ExternalInput")[:, :],
        "table_t": nc.dram_tensor("table_t", [C, cfg.shard], BF16,
                                  kind="ExternalInput")[:, :],
        "w": nc.dram_tensor("w", [K, C, C], BF16, kind="ExternalInput")[:, :, :],
        "gamma": nc.dram_tensor("gamma", [C], F32, kind="ExternalInput")[:],
        "beta": nc.dram_tensor("beta", [C], F32, kind="ExternalInput")[:],
    }
    if plans is None:
        ins["idx"] = nc.dram_tensor(
            "idx", [128, cfg.n_st * cfg.idx_cols], I16, kind="ExternalInput")[:, :]
    else:
        tot_icols = sum(pl.phat for pl in plans) // 16
        tot_chunks = sum(pl.n_chunks for pl in plans)
        ins["idx"] = nc.dram_tensor(
            "idx", [128, tot_icols], I16, kind="ExternalInput")[:, :]
        ins["slot"] = nc.dram_tensor(
            "slot", [128, tot_chunks], F32, kind="ExternalInput")[:, :]
        ins["iota"] = nc.dram_tensor(
            "iota", [128, cfg.st], F32, kind="ExternalInput")[:, :]
    out = nc.dram_tensor("out", [cfg.shard, C], F32, kind="ExternalOutput")
    with tile.TileContext(nc) as tc:
        for _ in range(reps):
            if plans is None:
                emit_dense(tc, out[:, :], ins, cfg)
            else:
                emit_sparse(tc, out[:, :], ins, cfg, plans)
    nc.compile()
    return nc


def prepare_all(features, W, nb, cfg):
    """Partition + plan + per-core input maps. Returns (core_maps, members,
    plans) with plans=None meaning the dense fallback program."""
    members, ok = _partition_components(nb, cfg.n_total, cfg.n_cores, cfg.shard)
    if not ok:
        return None, None, None
    plans, widths = build_plan(nb, members, cfg)
    if plans is not None:
        core_maps = _prepare_core_inputs_sparse(features, nb, members, cfg,
                                                plans, widths)
        iota = np.tile(np.arange(cfg.st, dtype=np.float32), (128, 1))
        extra = {"iota": iota}
    else:
        core_maps = _prepare_core_inputs_dense(features, nb, members, cfg)
        extra = {}
    w_bf = np.asarray(W, np.float32).astype(ml_dtypes.bfloat16)
    for m in core_maps:
        m["w"] = w_bf
        m.update(extra)
    return core_maps, members, plans


def kernel(features, W, b, gamma, beta, neighbor_idx):
    from concourse.bass_utils import run_bass_kernel_spmd

    features = np.asarray(features, np.float32)
    Wf = np.asarray(W, np.float32)
    gamma_f = np.asarray(gamma, np.float32)
    beta_f = np.asarray(beta, np.float32)
    nb = np.asarray(neighbor_idx, np.int32)
    cfg = FULL_CFG
    assert features.shape == (cfg.n_total, C)

    core_maps, members, plans = prepare_all(features, Wf, nb, cfg)
    if core_maps is None:
        return _reference_fallback(features, Wf, b, gamma_f, beta_f, nb)
    for m in core_maps:
        m["gamma"] = gamma_f
        m["beta"] = beta_f

    nc = _build_bass(cfg, plans=plans)
    res = run_bass_kernel_spmd(nc, core_maps, core_ids=list(range(cfg.n_cores)))

    out_full = np.empty((cfg.n_total, C), np.float32)
    for c, mem in enumerate(members):
        out_full[mem] = res.results[c]["out"][:len(mem)]
    return out_full
